# revision 25
# baseline (speedup 1.0000x reference)
"""Trainium2 Bass kernel for nn_MoE_32332513804634.

MoE: 16 routed experts (top-6, softmax-then-bias routing) + dense shared
expert, T=4096 tokens, D=2048, H=1408, HS=2816, fp32.

Strategy (8 NeuronCores, SPMD):
  - Host computes the gate (cheap) and per-expert token lists.
  - Expert parallelism as a per-core list of variable-width token chunks
    (width compiled in, identical multiset on every core; each chunk binds
    one expert's weights via its own dram tensors).
  - Precision split by combine weight: token-expert pairs with cw < TAU
    (~79% of routed compute) run as fp8-e4m3 chunks using DoubleRow
    matmuls (2 contraction planes per instr, ~1.5x PE throughput); the
    rest run fp16 (same speed as bf16, 4x lower quantization error).
    Weights for fp8 are pre-scaled by 64 on host; the 1/64 dequant folds
    into the activation scale (L1) and the per-token combine scale (L2).
  - Shared expert is tensor-parallel over its 2816 hidden dim (352 rows
    per core, padded to 384), fp16, weights SBUF-resident.
  - Host scatters chunk outputs back to token rows, sums partials, adds
    second-layer biases (cw*b2 per expert, bs2 once) in fp32.
"""

import sys
import numpy as np

sys.path.insert(0, "/opt/trn_rl_repo")

import concourse.bass as bass  # noqa: E402
import concourse.tile as tile  # noqa: E402
from concourse import bacc, mybir  # noqa: E402
from concourse.bass_utils import run_bass_kernel_spmd  # noqa: E402

T = 4096
D = 2048
H = 1408
E = 16
TOP_K = 6
HS = 2816
N_CORES = 8
HM = H // 128          # 11
KO = D // 128          # 16
HS_PAD = 384           # shared hidden shard (352) padded to 3*128
HMS = HS_PAD // 128    # 3
F32 = mybir.dt.float32
F16 = mybir.dt.float16
F8 = mybir.dt.float8e4
DR = mybir.MatmulPerfMode.DoubleRow

TAU = 0.15             # cw >= TAU pairs run fp16; below run fp8
W8SCALE = 64.0         # fp8 weight pre-scale (dequant folded downstream)

_PROGRAM_CACHE: dict = {}


def _np_f8():
    import ml_dtypes
    return ml_dtypes.float8_e4m3


def _host_gate(xf, gate_w, gate_b):
    scores = xf @ gate_w.T
    m = scores.max(axis=-1, keepdims=True)
    p = np.exp(scores - m, dtype=np.float32)
    probs = p / p.sum(axis=-1, keepdims=True)
    biased = probs + gate_b
    idx = np.argpartition(biased, E - TOP_K, axis=1)[:, E - TOP_K:]
    mask = np.zeros((xf.shape[0], E), dtype=bool)
    mask[np.arange(xf.shape[0])[:, None], idx] = True
    cw = np.where(mask, probs, 0.0).astype(np.float32)
    return cw, mask


def _chunk_cost(w):
    """Approx PE cost (ns) of one compiled fp16 chunk of width w."""
    l1 = 11 * 16 * 2 * max(107.0, w / 2.4 + 16)
    l2 = 4 * ((w + 127) // 128) * 11 * (512 / 2.4 + 16)
    return l1 + l2


def _mm_dr(w):
    """Empirical DoubleRow fp8 matmul issue spacing (ns) at moving width w
    (measured on HW: LDWEIGHTS partially unhidden below ~384)."""
    pts = [(128, 140.0), (256, 191.5), (320, 196.0), (384, 201.4),
           (448, 246.1), (512, 285.8)]
    if w <= pts[0][0]:
        return pts[0][1]
    for (w0, c0), (w1, c1) in zip(pts, pts[1:]):
        if w <= w1:
            return c0 + (c1 - c0) * (w - w0) / (w1 - w0)
    return pts[-1][1]


def _chunk_cost8(w):
    """Approx PE cost (ns) of one compiled fp8 chunk of width w."""
    l1 = 11 * 8 * 2 * _mm_dr(w)
    l2 = 4 * ((w + 127) // 128) * (5 * _mm_dr(512) + 304.0)
    return l1 + l2


def _cut_pieces(counts, target):
    """Cut each expert into near-equal pieces (each <= 512)."""
    pieces = []
    for e, c in enumerate(counts):
        c = int(c)
        if c == 0:
            continue
        k = max(1, -(-c // target))
        while -(-c // k) > 512:
            k += 1
        base, rem = divmod(c, k)
        start = 0
        for i in range(k):
            n = base + (1 if i < rem else 0)
            pieces.append((n, e, start))
            start += n
    return pieces


def _cut_pieces_base(counts, base_sz):
    """Cut into pieces of base_sz plus one ragged final piece per expert."""
    pieces = []
    for e, c in enumerate(counts):
        c = int(c)
        start = 0
        while c >= base_sz + 128:
            pieces.append((base_sz, e, start))
            start += base_sz
            c -= base_sz
        if c > 512:
            h1 = (c + 1) // 2
            pieces.append((h1, e, start))
            start += h1
            c -= h1
        if c > 0:
            pieces.append((c, e, start))
    return pieces


def _plan_groupsort(counts, cost_fn):
    """Equal-cut pieces, sorted and grouped 8-at-a-time into slots."""
    best = None
    cand = [_cut_pieces(counts, t) for t in range(320, 513, 8)]
    cand += [_cut_pieces_base(counts, b) for b in (512, 448, 384)]
    for pieces in cand:
        if not pieces:
            return (0.0, (), [[] for _ in range(N_CORES)])
        ps = sorted(pieces, key=lambda p: -p[0])
        nslots = -(-len(ps) // N_CORES)
        widths = []
        for s in range(nslots):
            grp = ps[s * N_CORES:(s + 1) * N_CORES]
            w = -(-max(p[0] for p in grp) // 16) * 16
            widths.append(w)
        cost = sum(cost_fn(w) for w in widths)
        if best is None or cost < best[0]:
            best = (cost, tuple(widths), ps)
    cost, widths, ps = best
    assignment = [[None] * len(widths) for _ in range(N_CORES)]
    for i, (n, e, st) in enumerate(ps):
        s, c = divmod(i, N_CORES)
        assignment[c][s] = (e, st, n)
    return cost, widths, assignment


def _solve_bundles3(nz, Ws, Is):
    """Exact DP: pick one (i, j, k) bundle per expert with per-width slot
    budgets Is. Returns list of (waste, (i, j, k)) per expert or None."""
    W1, W2, W3 = Ws
    I1, I2, I3 = Is
    opts = []
    for e, c in nz:
        o = []
        for i in range(0, min(I1, -(-c // W1)) + 1):
            r1 = c - i * W1
            jmax = min(I2, max(0, -(-r1 // W2))) if W2 else 0
            for j in range(0, jmax + 1):
                r2 = r1 - j * W2
                k = max(0, -(-r2 // W3)) if W3 else 0
                if k > I3 or (not W3 and r2 > 0):
                    continue
                o.append((i * W1 + j * W2 + k * W3 - c, (i, j, k)))
        if not o:
            return None
        opts.append(sorted(set(o)))
    reach = [np.zeros((I1 + 1, I2 + 1, I3 + 1), dtype=bool)]
    reach[0][0, 0, 0] = True
    for o in opts:
        cur = reach[-1]
        nxt = np.zeros_like(cur)
        for _, (i, j, k) in o:
            nxt[i:, j:, k:] |= cur[:I1 + 1 - i, :I2 + 1 - j, :I3 + 1 - k]
        if not nxt.any():
            return None
        reach.append(nxt)
    si, sj, sk = np.argwhere(reach[-1])[0]
    pick = [None] * len(opts)
    for idx in range(len(opts) - 1, -1, -1):
        for w, (i, j, k) in opts[idx]:
            if i <= si and j <= sj and k <= sk and \
                    reach[idx][si - i, sj - j, sk - k]:
                pick[idx] = (w, (i, j, k))
                si, sj, sk = si - i, sj - j, sk - k
                break
        if pick[idx] is None:
            return None
    return pick


def _plan_twowidth(counts, cost_fn):
    """Per-core multiset of up to 3 chunk widths; experts assigned slot
    bundles via exact DP; configs tried in ascending PE-cost order."""
    nz = [(e, int(c)) for e, c in enumerate(counts) if c > 0]
    if not nz:
        return None
    total = sum(c for _, c in nz)
    sizes = (512, 448, 384, 320, 256, 192, 128)
    configs = []
    seen = set()
    from itertools import combinations
    for ws in list(combinations(sizes, 2)) + list(combinations(sizes, 3)):
        W1, W2 = ws[0], ws[1]
        W3 = ws[2] if len(ws) == 3 else 0
        for a in range(0, 9):
            for b in range(0, 11):
                for cc in range(0, 9 if W3 else 1):
                    cap = a * W1 + b * W2 + cc * W3
                    if cap * N_CORES < total or cap > 4608:
                        continue
                    key = tuple(sorted([W1] * a + [W2] * b + [W3] * cc))
                    if key in seen:
                        continue
                    seen.add(key)
                    cost = (a * cost_fn(W1) + b * cost_fn(W2) +
                            (cc * cost_fn(W3) if W3 else 0))
                    configs.append((cost, W1, W2, W3, a, b, cc))
    configs.sort()
    best = None
    for cost, W1, W2, W3, a, b, cc in configs:
        pick = _solve_bundles3(nz, (W1, W2, W3),
                               (a * N_CORES, b * N_CORES, cc * N_CORES))
        if pick is not None:
            best = (cost, (W1, W2, W3, a, b, cc), pick)
            break
    if best is None:
        return None
    cost, (W1, W2, W3, a, b, cc), pick = best
    widths = (W1,) * a + (W2,) * b + ((W3,) * cc if W3 else ())
    slots = [[(core, s) for s in range(a) for core in range(N_CORES)],
             [(core, s + a) for s in range(b) for core in range(N_CORES)],
             [(core, s + a + b) for s in range(cc) for core in range(N_CORES)]]
    ptr = [0, 0, 0]
    Wv = (W1, W2, W3)
    assignment = [[None] * len(widths) for _ in range(N_CORES)]
    for (e, c), (w, ijk) in zip(nz, pick):
        start = 0
        rem = c
        for lvl in range(3):
            for _ in range(ijk[lvl]):
                core, s = slots[lvl][ptr[lvl]]; ptr[lvl] += 1
                n = min(rem, Wv[lvl])
                if n > 0:
                    assignment[core][s] = (e, start, n)
                start += n
                rem -= n
        assert rem == 0, (e, c, rem)
    return cost, widths, assignment


def _plan(counts, cost_fn):
    """Returns (widths, assignment): widths = per-core compiled chunk
    widths; assignment[core][slot] = (expert, start, fill) or None."""
    plans = [_plan_groupsort(counts, cost_fn)]
    tw = _plan_twowidth(counts, cost_fn)
    if tw is not None:
        plans.append(tw)
    plans.sort(key=lambda p: p[0])
    _, widths, assignment = plans[0]
    return tuple(widths), assignment


def _build_program(widths8, widths16):
    nc = bacc.Bacc("TRN2", debug=False, num_devices=N_CORES)

    ins = {}
    outs = {}

    def din(name, shape, dt):
        ins[name] = nc.dram_tensor(name, list(shape), dt, kind="ExternalInput").ap()
        return ins[name]

    def dout(name, shape, dt=F16):
        outs[name] = nc.dram_tensor(name, list(shape), dt, kind="ExternalOutput").ap()
        return outs[name]

    # All inputs are pre-tiled on host so every DMA reads large contiguous
    # per-partition spans (the naive [D, H] layouts produced 128-256B DMA
    # packets that saturated the DMA engines' packet rate).
    for s, w in enumerate(widths8):
        ntch = -(-w // 128)
        din(f"xg8_{s}", (128, KO, w), F8)
        din(f"w1t8_{s}", (HM, 128, KO, 128), F8)
        din(f"w3t8_{s}", (HM, 128, KO, 128), F8)
        din(f"w2t8_{s}", (4, 128, HM, 512), F8)
        din(f"b1_8_{s}", (128, HM), F32)
        din(f"b3_8_{s}", (128, HM), F32)
        din(f"scl8_{s}", (128, ntch), F32)
        dout(f"oe8_{s}", (ntch * 128, D))
    for s, w in enumerate(widths16):
        ntch = -(-w // 128)
        din(f"xg16_{s}", (128, KO, w), F16)
        din(f"w1t16_{s}", (HM, 128, KO, 128), F16)
        din(f"w3t16_{s}", (HM, 128, KO, 128), F16)
        din(f"w2t16_{s}", (4, 128, HM, 512), F16)
        din(f"b1_16_{s}", (128, HM), F32)
        din(f"b3_16_{s}", (128, HM), F32)
        din(f"scl16_{s}", (128, ntch), F32)
        dout(f"oe16_{s}", (ntch * 128, D))
    # shared expert: hidden-sharded (352 -> pad 384 per core), weights
    # SBUF-resident, 8 light token chunks interleaved as DMA relief
    din("xt", (T // 512, 128, KO, 512), F16)
    din("ws1s", (128, KO, HS_PAD), F16)
    din("ws3s", (128, KO, HS_PAD), F16)
    din("ws2sa", (128, HMS, D), F16)
    din("bs1", (128, HMS), F32)
    din("bs3", (128, HMS), F32)
    dout("zs", (T, D))

    with tile.TileContext(nc) as tc:
        with (
            tc.tile_pool(name="xpool", bufs=2) as xpool,
            tc.tile_pool(name="hpool", bufs=2) as hpool,
            tc.tile_pool(name="wcol", bufs=3) as wcol,
            tc.tile_pool(name="w2pool", bufs=2) as w2pool,
            tc.tile_pool(name="tmp", bufs=2) as tmp,
            tc.tile_pool(name="opool", bufs=6) as opool,
            tc.tile_pool(name="cpool", bufs=1) as cpool,
            tc.tile_pool(name="pp", bufs=2, space="PSUM") as pp,
        ):
            def dma_planes(dst, src, nplanes, group):
                """Split a [128, planes, w] tile DMA into per-partition runs
                of `group` planes: big enough to beat the DMA engines'
                packet-rate bound, small enough not to stall PE SBUF reads."""
                for q0 in range(0, nplanes, group):
                    q1 = min(nplanes, q0 + group)
                    nc.sync.dma_start(dst[:, q0:q1], src[:, q0:q1])

            def prefetch_x8(s, w):
                xsb = xpool.tile([128, KO, 512], F8, tag="xg8")
                dma_planes(xsb[:, :, :w], ins[f"xg8_{s}"], KO, 2)
                return xsb

            def prefetch_x16(s, w):
                xsb = xpool.tile([128, KO, 512], F16, tag="xg16")
                dma_planes(xsb[:, :, :w], ins[f"xg16_{s}"], KO, 2)
                return xsb

            def prefetch_xs(t):
                xsb = xpool.tile([128, KO, 512], F16, tag="xg16")
                dma_planes(xsb[:], ins["xt"][t], KO, 2)
                return xsb

            def mlp_chunk8(s, w, xsb):
                """fp8 chunk: out[:w] = scl * (swiglu8(xg) @ W2^T); weights
                pre-scaled x64, dequant via act scale + scl (=cw/64)."""
                ntch = -(-w // 128)
                w1c3 = ins[f"w1t8_{s}"]
                w3c3 = ins[f"w3t8_{s}"]
                w23 = ins[f"w2t8_{s}"]
                out_ap = outs[f"oe8_{s}"]

                b1sb = cpool.tile([128, HM], F32, tag=f"b1f8{s}")
                nc.sync.dma_start(b1sb[:], ins[f"b1_8_{s}"])
                b3sb = cpool.tile([128, HM], F32, tag=f"b3f8{s}")
                nc.sync.dma_start(b3sb[:], ins[f"b3_8_{s}"])
                sclsb = cpool.tile([128, ntch], F32, tag=f"sclf8{s}")
                nc.sync.dma_start(sclsb[:], ins[f"scl8_{s}"])

                w2sbs = []
                hsb = hpool.tile([128, HM, 512], F8, tag="h8", bufs=2)
                for hm in range(HM):
                    if hm in (3, 8):
                        dm = 0 if hm == 3 else 1
                        w2sb = w2pool.tile([128, HM, 512], F8, tag="w2s8",
                                           bufs=2)
                        dma_planes(w2sb[:], w23[dm], HM, 2)
                        w2sbs.append(w2sb)
                    w1t_ = wcol.tile([128, KO, 128], F8, tag="w1c8", bufs=3)
                    dma_planes(w1t_[:], w1c3[hm], KO, 4)
                    w3t_ = wcol.tile([128, KO, 128], F8, tag="w3c8", bufs=3)
                    dma_planes(w3t_[:], w3c3[hm], KO, 4)
                    ps1 = pp.tile([128, 512], F32, tag="ph1")
                    for kk in range(KO // 2):
                        nc.tensor.matmul(ps1[:, :w],
                                         w1t_[:, 2 * kk:2 * kk + 2, :],
                                         xsb[:, 2 * kk:2 * kk + 2, :w],
                                         start=(kk == 0), stop=(kk == KO // 2 - 1),
                                         perf_mode=DR)
                    ps3 = pp.tile([128, 512], F32, tag="ph3")
                    for kk in range(KO // 2):
                        nc.tensor.matmul(ps3[:, :w],
                                         w3t_[:, 2 * kk:2 * kk + 2, :],
                                         xsb[:, 2 * kk:2 * kk + 2, :w],
                                         start=(kk == 0), stop=(kk == KO // 2 - 1),
                                         perf_mode=DR)
                    h1t = tmp.tile([128, 512], F32, tag="h1t")
                    nc.scalar.activation(h1t[:, :w], ps1[:, :w],
                                         mybir.ActivationFunctionType.Silu,
                                         bias=b1sb[:, hm:hm + 1],
                                         scale=1.0 / W8SCALE)
                    h3t = tmp.tile([128, 512], F32, tag="h3t")
                    nc.scalar.activation(h3t[:, :w], ps3[:, :w],
                                         mybir.ActivationFunctionType.Identity,
                                         bias=b3sb[:, hm:hm + 1],
                                         scale=1.0 / W8SCALE)
                    nc.vector.tensor_mul(hsb[:, hm, :w], h1t[:, :w], h3t[:, :w])
                # second matmul: out rows = tokens; 5 DoubleRow + 1 single
                for dm in range(4):
                    if dm == 1:
                        w2sb = w2pool.tile([128, HM, 512], F8, tag="w2s8",
                                           bufs=2)
                        dma_planes(w2sb[:], w23[2], HM, 2)
                        w2sbs.append(w2sb)
                    if dm == 2:
                        w2sb = w2pool.tile([128, HM, 512], F8, tag="w2s8",
                                           bufs=2)
                        dma_planes(w2sb[:], w23[3], HM, 2)
                        w2sbs.append(w2sb)
                    w2sb = w2sbs[dm]
                    for tch in range(ntch):
                        tok0 = tch * 128
                        tcw = min(128, w - tok0)
                        ps2 = pp.tile([128, 512], F32, tag="po", bufs=4)
                        for kk in range(5):
                            nc.tensor.matmul(
                                ps2[:tcw, :],
                                hsb[:, 2 * kk:2 * kk + 2, tok0:tok0 + tcw],
                                w2sb[:, 2 * kk:2 * kk + 2, :],
                                start=(kk == 0), stop=False, perf_mode=DR)
                        nc.tensor.matmul(ps2[:tcw, :],
                                         hsb[:, 10, tok0:tok0 + tcw],
                                         w2sb[:, 10, :],
                                         start=False, stop=True)
                        osb = opool.tile([128, 512], F16, tag="osb")
                        nc.vector.tensor_scalar_mul(
                            osb[:tcw, :], ps2[:tcw, :], sclsb[:tcw, tch:tch + 1])
                        nc.sync.dma_start(
                            out_ap[tok0:tok0 + tcw, dm * 512:(dm + 1) * 512],
                            osb[:tcw, :])

            def mlp_chunk16(xsb, w1_ap, w3_ap, w2_ap, b1_ap, b3_ap, scl_ap,
                            out_ap, w, n_hm, wtag):
                """fp16 chunk: out[:w] = scale * (swiglu(xg) @ W2^T)."""
                ntch = -(-w // 128)
                w1c3 = w1_ap
                w3c3 = w3_ap
                w23 = w2_ap

                b1sb = cpool.tile([128, n_hm], F32, tag=f"b1{wtag}")
                nc.sync.dma_start(b1sb[:], b1_ap)
                b3sb = cpool.tile([128, n_hm], F32, tag=f"b3{wtag}")
                nc.sync.dma_start(b3sb[:], b3_ap)
                if scl_ap is not None:
                    sclsb = cpool.tile([128, ntch], F32, tag=f"scl{wtag}")
                    nc.sync.dma_start(sclsb[:], scl_ap)

                w2sbs = []
                hsb = hpool.tile([128, n_hm, 512], F16, tag=f"h16_{n_hm}",
                                 bufs=2)
                for hm in range(n_hm):
                    if hm in (3, 8):
                        dm = 0 if hm == 3 else 1
                        w2sb = w2pool.tile([128, n_hm, 512], F16,
                                           tag=f"w2s16_{n_hm}", bufs=2)
                        dma_planes(w2sb[:], w23[dm], HM, 2)
                        w2sbs.append(w2sb)
                    w1t_ = wcol.tile([128, KO, 128], F16, tag="w1c16", bufs=3)
                    dma_planes(w1t_[:], w1c3[hm], KO, 4)
                    w3t_ = wcol.tile([128, KO, 128], F16, tag="w3c16", bufs=3)
                    dma_planes(w3t_[:], w3c3[hm], KO, 4)
                    ps1 = pp.tile([128, 512], F32, tag="ph1")
                    for ko in range(KO):
                        nc.tensor.matmul(ps1[:, :w], w1t_[:, ko, :], xsb[:, ko, :w],
                                         start=(ko == 0), stop=(ko == KO - 1))
                    ps3 = pp.tile([128, 512], F32, tag="ph3")
                    for ko in range(KO):
                        nc.tensor.matmul(ps3[:, :w], w3t_[:, ko, :], xsb[:, ko, :w],
                                         start=(ko == 0), stop=(ko == KO - 1))
                    h1t = tmp.tile([128, 512], F32, tag="h1t")
                    nc.scalar.activation(h1t[:, :w], ps1[:, :w],
                                         mybir.ActivationFunctionType.Silu,
                                         bias=b1sb[:, hm:hm + 1])
                    h3t = tmp.tile([128, 512], F32, tag="h3t")
                    nc.scalar.activation(h3t[:, :w], ps3[:, :w],
                                         mybir.ActivationFunctionType.Identity,
                                         bias=b3sb[:, hm:hm + 1])
                    nc.vector.tensor_mul(hsb[:, hm, :w], h1t[:, :w], h3t[:, :w])
                # second matmul: out rows = tokens
                for dm in range(4):
                    if n_hm > 4 and dm in (1, 2):
                        w2sb = w2pool.tile([128, n_hm, 512], F16,
                                           tag=f"w2s16_{n_hm}", bufs=2)
                        dma_planes(w2sb[:], w23[dm + 1], HM, 2)
                        w2sbs.append(w2sb)
                    w2sb = w2sbs[dm]
                    for tch in range(ntch):
                        tok0 = tch * 128
                        tcw = min(128, w - tok0)
                        ps2 = pp.tile([128, 512], F32, tag="po", bufs=4)
                        for k in range(n_hm):
                            lhsT = hsb[:, k, tok0:tok0 + tcw]
                            nc.tensor.matmul(ps2[:tcw, :], lhsT, w2sb[:, k, :],
                                             start=(k == 0), stop=(k == n_hm - 1))
                        osb = opool.tile([128, 512], F16, tag="osb")
                        if scl_ap is not None:
                            nc.vector.tensor_scalar_mul(
                                osb[:tcw, :], ps2[:tcw, :], sclsb[:tcw, tch:tch + 1])
                        else:
                            nc.vector.tensor_copy(osb[:tcw, :], ps2[:tcw, :])
                        nc.sync.dma_start(
                            out_ap[tok0:tok0 + tcw, dm * 512:(dm + 1) * 512],
                            osb[:tcw, :])

            # shared-expert residents (emitted piecewise between early chunks)
            ws1r = cpool.tile([128, KO, HS_PAD], F16, tag="ws1r")
            ws3r = cpool.tile([128, KO, HS_PAD], F16, tag="ws3r")
            ws2r = cpool.tile([128, HMS, D], F16, tag="ws2r")
            bs1r = cpool.tile([128, HMS], F32, tag="bs1r")
            bs3r = cpool.tile([128, HMS], F32, tag="bs3r")
            resident_loads = [
                lambda: dma_planes(ws1r[:], ins["ws1s"], KO, 2),
                lambda: dma_planes(ws3r[:], ins["ws3s"], KO, 2),
                lambda: (dma_planes(ws2r[:], ins["ws2sa"], HMS, 1),
                         nc.sync.dma_start(bs1r[:], ins["bs1"]),
                         nc.sync.dma_start(bs3r[:], ins["bs3"])),
            ]

            def routed_chunk16(s, w, xsb):
                mlp_chunk16(xsb, ins[f"w1t16_{s}"], ins[f"w3t16_{s}"],
                            ins[f"w2t16_{s}"], ins[f"b1_16_{s}"], ins[f"b3_16_{s}"],
                            ins[f"scl16_{s}"], outs[f"oe16_{s}"], w, HM, f"e{s}")

            def shared_chunk(t, xsb):
                hsb = hpool.tile([128, HM, 512], F16, tag=f"h16_{HM}", bufs=2)
                for hm in range(HMS):
                    ps1 = pp.tile([128, 512], F32, tag="ph1")
                    for ko in range(KO):
                        nc.tensor.matmul(ps1[:], ws1r[:, ko, hm * 128:(hm + 1) * 128],
                                         xsb[:, ko, :],
                                         start=(ko == 0), stop=(ko == KO - 1))
                    ps3 = pp.tile([128, 512], F32, tag="ph3")
                    for ko in range(KO):
                        nc.tensor.matmul(ps3[:], ws3r[:, ko, hm * 128:(hm + 1) * 128],
                                         xsb[:, ko, :],
                                         start=(ko == 0), stop=(ko == KO - 1))
                    h1t = tmp.tile([128, 512], F32, tag="h1t")
                    nc.scalar.activation(h1t[:], ps1[:],
                                         mybir.ActivationFunctionType.Silu,
                                         bias=bs1r[:, hm:hm + 1])
                    h3t = tmp.tile([128, 512], F32, tag="h3t")
                    nc.scalar.activation(h3t[:], ps3[:],
                                         mybir.ActivationFunctionType.Identity,
                                         bias=bs3r[:, hm:hm + 1])
                    nc.vector.tensor_mul(hsb[:, hm, :], h1t[:], h3t[:])
                for dm in range(4):
                    for tch in range(4):
                        tok0 = t * 512 + tch * 128
                        ps2 = pp.tile([128, 512], F32, tag="po", bufs=4)
                        for k in range(HMS):
                            lhsT = hsb[:, k, tch * 128:(tch + 1) * 128]
                            nc.tensor.matmul(ps2[:], lhsT,
                                             ws2r[:, k, dm * 512:(dm + 1) * 512],
                                             start=(k == 0), stop=(k == HMS - 1))
                        osb = opool.tile([128, 512], F16, tag="osb")
                        nc.vector.tensor_copy(osb[:], ps2[:])
                        nc.sync.dma_start(
                            outs["zs"][tok0:tok0 + 128, dm * 512:(dm + 1) * 512],
                            osb[:])

            # routed order: spread fp16 chunks evenly among fp8 chunks
            n8, n16 = len(widths8), len(widths16)
            routed = []
            i8 = i16 = 0
            while i8 < n8 or i16 < n16:
                if i8 < n8:
                    routed.append(("r8", i8)); i8 += 1
                while i16 < n16 and (i8 >= n8 or (i16 + 1) * n8 <= (i8) * n16 + n16 // 2):
                    routed.append(("r16", i16)); i16 += 1
            # interleave shared chunks among routed[2:] (delay past the
            # resident loads + warmup to avoid the early DMA burst stall)
            n_r, n_s = len(routed), T // 512
            lead = min(2, n_r)
            seq = routed[:lead]
            ri, si = lead, 0
            nr_rem = n_r - lead
            while ri < n_r or si < n_s:
                if ri < n_r:
                    seq.append(routed[ri]); ri += 1
                while si < n_s and (ri >= n_r or
                                    (si + 1) * nr_rem <= (ri - lead + 1) * n_s):
                    seq.append(("s", si)); si += 1

            def prefetch(item):
                kind, i = item
                if kind == "r8":
                    return prefetch_x8(i, widths8[i])
                if kind == "r16":
                    return prefetch_x16(i, widths16[i])
                return prefetch_xs(i)

            pending = prefetch(seq[0])
            for n_emitted, (kind, i) in enumerate(seq):
                xsb_cur = pending
                if n_emitted + 1 < len(seq):
                    pending = prefetch(seq[n_emitted + 1])
                if kind == "r8":
                    mlp_chunk8(i, widths8[i], xsb_cur)
                elif kind == "r16":
                    routed_chunk16(i, widths16[i], xsb_cur)
                else:
                    shared_chunk(i, xsb_cur)
                # residents loaded during the first two chunks
                if n_emitted == 0:
                    resident_loads[0]()
                if n_emitted == min(1, len(seq) - 1):
                    resident_loads[1]()
                    resident_loads[2]()

    nc.compile()
    return nc


def kernel(x, gate_w, gate_b, w1, b1, w2, b2, w3, b3,
           ws1, bs1, ws2, bs2, ws3, bs3):
    F8NP = _np_f8()
    x = np.asarray(x, np.float32)
    xf = np.ascontiguousarray(x.reshape(-1, D))
    gate_w = np.asarray(gate_w, np.float32)
    gate_b = np.asarray(gate_b, np.float32)
    w1 = np.asarray(w1, np.float32)
    b1 = np.asarray(b1, np.float32)
    w2 = np.asarray(w2, np.float32)
    b2 = np.asarray(b2, np.float32)
    w3 = np.asarray(w3, np.float32)
    b3 = np.asarray(b3, np.float32)
    ws1 = np.asarray(ws1, np.float32)
    bs1 = np.asarray(bs1, np.float32)
    ws2 = np.asarray(ws2, np.float32)
    bs2 = np.asarray(bs2, np.float32)
    ws3 = np.asarray(ws3, np.float32)
    bs3 = np.asarray(bs3, np.float32)

    cw, mask = _host_gate(xf, gate_w, gate_b)
    hi = mask & (cw >= TAU)
    lo = mask & (cw < TAU)
    toks8 = [np.flatnonzero(lo[:, e]).astype(np.int64) for e in range(E)]
    toks16 = [np.flatnonzero(hi[:, e]).astype(np.int64) for e in range(E)]
    counts8 = np.array([len(t) for t in toks8])
    counts16 = np.array([len(t) for t in toks16])
    widths8, asg8 = _plan(counts8, _chunk_cost8)
    widths16, asg16 = _plan(counts16, _chunk_cost)

    key = (widths8, widths16)
    if key not in _PROGRAM_CACHE:
        _PROGRAM_CACHE[key] = _build_program(widths8, widths16)
    nc = _PROGRAM_CACHE[key]

    xT = np.ascontiguousarray(xf.T)  # [D, T]
    xT16 = xT.astype(np.float16)
    xT8 = xT.astype(F8NP)

    def tile_w13(a):
        # [D, H] -> [HM, 128, KO, 128]: per-partition-contiguous tiles
        return np.ascontiguousarray(
            a.reshape(KO, 128, HM, 128).transpose(2, 1, 0, 3))

    def tile_w2(a):
        # [H, D] -> [4, 128, HM, 512]
        return np.ascontiguousarray(
            a.reshape(HM, 128, 4, 512).transpose(2, 1, 0, 3))

    w1t8, w3t8, w2t8 = {}, {}, {}
    w1t16, w3t16, w2t16 = {}, {}, {}
    b1t, b3t = {}, {}

    def prep_bias(e):
        if e not in b1t:
            b1t[e] = np.ascontiguousarray(b1[e].reshape(HM, 128).T)
            b3t[e] = np.ascontiguousarray(b3[e].reshape(HM, 128).T)

    need8 = sorted({p[0] for slots in asg8 for p in slots if p is not None})
    need16 = sorted({p[0] for slots in asg16 for p in slots if p is not None})
    for e in need8:
        w1t8[e] = tile_w13((w1[e].T * W8SCALE).astype(F8NP))
        w3t8[e] = tile_w13((w3[e].T * W8SCALE).astype(F8NP))
        w2t8[e] = tile_w2((w2[e].T * W8SCALE).astype(F8NP))
        prep_bias(e)
    for e in need16:
        w1t16[e] = tile_w13(w1[e].T.astype(np.float16))
        w3t16[e] = tile_w13(w3[e].T.astype(np.float16))
        w2t16[e] = tile_w2(w2[e].T.astype(np.float16))
        prep_bias(e)

    hs_per = HS // N_CORES  # 352
    # [T//512, 128, KO, 512] pre-tiled shared-x (per-partition contiguous)
    xt_tiled = np.ascontiguousarray(
        xT16.reshape(KO, 128, T // 512, 512).transpose(2, 1, 0, 3))

    in_maps = []
    for c in range(N_CORES):
        m = {}
        for s, w in enumerate(widths8):
            ntch = -(-w // 128)
            piece = asg8[c][s]
            xg = np.zeros((D, w), F8NP)
            scl = np.zeros(ntch * 128, np.float32)
            if piece is None:
                e = need8[0]
            else:
                e, s0, n = piece
                tk = toks8[e][s0:s0 + n]
                xg[:, :n] = xT8[:, tk]
                scl[:n] = cw[tk, e] / W8SCALE
            m[f"w1t8_{s}"] = w1t8[e]
            m[f"w3t8_{s}"] = w3t8[e]
            m[f"w2t8_{s}"] = w2t8[e]
            m[f"b1_8_{s}"] = b1t[e]
            m[f"b3_8_{s}"] = b3t[e]
            m[f"xg8_{s}"] = np.ascontiguousarray(
                xg.reshape(KO, 128, w).transpose(1, 0, 2))
            m[f"scl8_{s}"] = np.ascontiguousarray(scl.reshape(ntch, 128).T)
        for s, w in enumerate(widths16):
            ntch = -(-w // 128)
            piece = asg16[c][s]
            xg = np.zeros((D, w), np.float16)
            scl = np.zeros(ntch * 128, np.float32)
            if piece is None:
                e = need16[0]
            else:
                e, s0, n = piece
                tk = toks16[e][s0:s0 + n]
                xg[:, :n] = xT16[:, tk]
                scl[:n] = cw[tk, e]
            m[f"w1t16_{s}"] = w1t16[e]
            m[f"w3t16_{s}"] = w3t16[e]
            m[f"w2t16_{s}"] = w2t16[e]
            m[f"b1_16_{s}"] = b1t[e]
            m[f"b3_16_{s}"] = b3t[e]
            m[f"xg16_{s}"] = np.ascontiguousarray(
                xg.reshape(KO, 128, w).transpose(1, 0, 2))
            m[f"scl16_{s}"] = np.ascontiguousarray(scl.reshape(ntch, 128).T)
        # shared expert shard (352 hidden rows, padded to 384)
        r0 = c * hs_per
        ws1p = np.zeros((D, HS_PAD), np.float16)
        ws1p[:, :hs_per] = ws1[r0:r0 + hs_per].T
        ws3p = np.zeros((D, HS_PAD), np.float16)
        ws3p[:, :hs_per] = ws3[r0:r0 + hs_per].T
        ws2a = np.zeros((HS_PAD, D), np.float16)
        ws2a[:hs_per] = ws2[:, r0:r0 + hs_per].T
        bs1p = np.zeros(HS_PAD, np.float32)
        bs1p[:hs_per] = bs1[r0:r0 + hs_per]
        bs3p = np.zeros(HS_PAD, np.float32)
        bs3p[:hs_per] = bs3[r0:r0 + hs_per]
        m["xt"] = xt_tiled
        m["ws1s"] = np.ascontiguousarray(
            ws1p.reshape(KO, 128, HS_PAD).transpose(1, 0, 2))
        m["ws3s"] = np.ascontiguousarray(
            ws3p.reshape(KO, 128, HS_PAD).transpose(1, 0, 2))
        m["ws2sa"] = np.ascontiguousarray(
            ws2a.reshape(HMS, 128, D).transpose(1, 0, 2))
        m["bs1"] = np.ascontiguousarray(bs1p.reshape(HMS, 128).T)
        m["bs3"] = np.ascontiguousarray(bs3p.reshape(HMS, 128).T)
        in_maps.append(m)

    res = run_bass_kernel_spmd(nc, in_maps, list(range(N_CORES)))

    y = np.zeros((T, D), np.float32)
    for c in range(N_CORES):
        for s, w in enumerate(widths8):
            piece = asg8[c][s]
            if piece is None:
                continue
            e, s0, n = piece
            tk = toks8[e][s0:s0 + n]
            y[tk] += res.results[c][f"oe8_{s}"][:n].astype(np.float32)
            y[tk] += cw[tk, e][:, None] * b2[e][None, :]
        for s, w in enumerate(widths16):
            piece = asg16[c][s]
            if piece is None:
                continue
            e, s0, n = piece
            tk = toks16[e][s0:s0 + n]
            y[tk] += res.results[c][f"oe16_{s}"][:n].astype(np.float32)
            y[tk] += cw[tk, e][:, None] * b2[e][None, :]
        y += res.results[c]["zs"].astype(np.float32)
    y += bs2[None, :]
    return y.reshape(x.shape).astype(np.float32)


# revision 28
# speedup vs baseline: 1.2270x; 1.2270x over previous
"""Trainium2 Bass kernel for nn_MoE_32332513804634.

MoE: 16 routed experts (top-6, softmax-then-bias routing) + dense shared
expert, T=4096 tokens, D=2048, H=1408, HS=2816, fp32.

Strategy (8 NeuronCores, SPMD):
  - Host computes the gate (cheap) and per-expert token lists.
  - Expert parallelism as a per-core list of variable-width token chunks
    (width compiled in, identical multiset on every core; each chunk binds
    one expert's weights via its own dram tensors).
  - Precision split by combine weight: token-expert pairs with cw < TAU
    (~79% of routed compute) run as fp8-e4m3 chunks using DoubleRow
    matmuls (2 contraction planes per instr, ~1.5x PE throughput); the
    rest run fp16 (same speed as bf16, 4x lower quantization error).
    Weights for fp8 are pre-scaled by 64 on host; the 1/64 dequant folds
    into the activation scale (L1) and the per-token combine scale (L2).
  - Shared expert is tensor-parallel over its 2816 hidden dim (352 rows
    per core, padded to 384), fp16, weights SBUF-resident.
  - Host scatters chunk outputs back to token rows, sums partials, adds
    second-layer biases (cw*b2 per expert, bs2 once) in fp32.
"""

import sys
import numpy as np

sys.path.insert(0, "/opt/trn_rl_repo")

import concourse.bass as bass  # noqa: E402
import concourse.tile as tile  # noqa: E402
from concourse import bacc, mybir  # noqa: E402
from concourse.bass_utils import run_bass_kernel_spmd  # noqa: E402

T = 4096
D = 2048
H = 1408
E = 16
TOP_K = 6
HS = 2816
N_CORES = 8
HM = H // 128          # 11
KO = D // 128          # 16
HS_PAD = 384           # shared hidden shard (352) padded to 3*128
HMS = HS_PAD // 128    # 3
F32 = mybir.dt.float32
F16 = mybir.dt.float16
F8 = mybir.dt.float8e4
DR = mybir.MatmulPerfMode.DoubleRow

TAU = 0.15             # cw >= TAU pairs run fp16; below run fp8
W8SCALE = 64.0         # fp8 weight pre-scale (dequant folded downstream)

_PROGRAM_CACHE: dict = {}


def _np_f8():
    import ml_dtypes
    return ml_dtypes.float8_e4m3


def _host_gate(xf, gate_w, gate_b):
    scores = xf @ gate_w.T
    m = scores.max(axis=-1, keepdims=True)
    p = np.exp(scores - m, dtype=np.float32)
    probs = p / p.sum(axis=-1, keepdims=True)
    biased = probs + gate_b
    idx = np.argpartition(biased, E - TOP_K, axis=1)[:, E - TOP_K:]
    mask = np.zeros((xf.shape[0], E), dtype=bool)
    mask[np.arange(xf.shape[0])[:, None], idx] = True
    cw = np.where(mask, probs, 0.0).astype(np.float32)
    return cw, mask


def _chunk_cost(w):
    """Approx PE cost (ns) of one compiled fp16 chunk of width w."""
    l1 = 11 * 16 * 2 * max(107.0, w / 2.4 + 16)
    l2 = 4 * ((w + 127) // 128) * 11 * (512 / 2.4 + 16)
    return l1 + l2


def _chunk_cost8(w):
    """Approx PE cost (ns) of one compiled fp8 chunk of width w."""
    l1 = 11 * 8 * 2 * max(120.0, w * 0.578 + 16)
    l2 = 4 * ((w + 127) // 128) * (5 * (512 * 0.578 + 16) + (512 / 2.4 + 16))
    return l1 + l2


def _cut_pieces(counts, target):
    """Cut each expert into near-equal pieces (each <= 512)."""
    pieces = []
    for e, c in enumerate(counts):
        c = int(c)
        if c == 0:
            continue
        k = max(1, -(-c // target))
        while -(-c // k) > 512:
            k += 1
        base, rem = divmod(c, k)
        start = 0
        for i in range(k):
            n = base + (1 if i < rem else 0)
            pieces.append((n, e, start))
            start += n
    return pieces


def _cut_pieces_base(counts, base_sz):
    """Cut into pieces of base_sz plus one ragged final piece per expert."""
    pieces = []
    for e, c in enumerate(counts):
        c = int(c)
        start = 0
        while c >= base_sz + 128:
            pieces.append((base_sz, e, start))
            start += base_sz
            c -= base_sz
        if c > 512:
            h1 = (c + 1) // 2
            pieces.append((h1, e, start))
            start += h1
            c -= h1
        if c > 0:
            pieces.append((c, e, start))
    return pieces


def _plan_groupsort(counts, cost_fn):
    """Equal-cut pieces, sorted and grouped 8-at-a-time into slots."""
    best = None
    cand = [_cut_pieces(counts, t) for t in range(320, 513, 8)]
    cand += [_cut_pieces_base(counts, b) for b in (512, 448, 384)]
    for pieces in cand:
        if not pieces:
            return (0.0, (), [[] for _ in range(N_CORES)])
        ps = sorted(pieces, key=lambda p: -p[0])
        nslots = -(-len(ps) // N_CORES)
        widths = []
        for s in range(nslots):
            grp = ps[s * N_CORES:(s + 1) * N_CORES]
            w = -(-max(p[0] for p in grp) // 16) * 16
            widths.append(w)
        cost = sum(cost_fn(w) for w in widths)
        if best is None or cost < best[0]:
            best = (cost, tuple(widths), ps)
    cost, widths, ps = best
    assignment = [[None] * len(widths) for _ in range(N_CORES)]
    for i, (n, e, st) in enumerate(ps):
        s, c = divmod(i, N_CORES)
        assignment[c][s] = (e, st, n)
    return cost, widths, assignment


def _solve_bundles3(nz, Ws, Is):
    """Exact DP: pick one (i, j, k) bundle per expert with per-width slot
    budgets Is. Returns list of (waste, (i, j, k)) per expert or None."""
    W1, W2, W3 = Ws
    I1, I2, I3 = Is
    opts = []
    for e, c in nz:
        o = []
        for i in range(0, min(I1, -(-c // W1)) + 1):
            r1 = c - i * W1
            jmax = min(I2, max(0, -(-r1 // W2))) if W2 else 0
            for j in range(0, jmax + 1):
                r2 = r1 - j * W2
                k = max(0, -(-r2 // W3)) if W3 else 0
                if k > I3 or (not W3 and r2 > 0):
                    continue
                o.append((i * W1 + j * W2 + k * W3 - c, (i, j, k)))
        if not o:
            return None
        opts.append(sorted(set(o)))
    reach = [np.zeros((I1 + 1, I2 + 1, I3 + 1), dtype=bool)]
    reach[0][0, 0, 0] = True
    for o in opts:
        cur = reach[-1]
        nxt = np.zeros_like(cur)
        for _, (i, j, k) in o:
            nxt[i:, j:, k:] |= cur[:I1 + 1 - i, :I2 + 1 - j, :I3 + 1 - k]
        if not nxt.any():
            return None
        reach.append(nxt)
    si, sj, sk = np.argwhere(reach[-1])[0]
    pick = [None] * len(opts)
    for idx in range(len(opts) - 1, -1, -1):
        for w, (i, j, k) in opts[idx]:
            if i <= si and j <= sj and k <= sk and \
                    reach[idx][si - i, sj - j, sk - k]:
                pick[idx] = (w, (i, j, k))
                si, sj, sk = si - i, sj - j, sk - k
                break
        if pick[idx] is None:
            return None
    return pick


def _plan_twowidth(counts, cost_fn):
    """Per-core multiset of up to 3 chunk widths; experts assigned slot
    bundles via exact DP; configs tried in ascending PE-cost order."""
    nz = [(e, int(c)) for e, c in enumerate(counts) if c > 0]
    if not nz:
        return None
    total = sum(c for _, c in nz)
    sizes = (512, 448, 384, 320, 256, 192, 128)
    configs = []
    seen = set()
    from itertools import combinations
    for ws in list(combinations(sizes, 2)) + list(combinations(sizes, 3)):
        W1, W2 = ws[0], ws[1]
        W3 = ws[2] if len(ws) == 3 else 0
        for a in range(0, 9):
            for b in range(0, 11):
                for cc in range(0, 9 if W3 else 1):
                    cap = a * W1 + b * W2 + cc * W3
                    if cap * N_CORES < total or cap > 4608:
                        continue
                    key = tuple(sorted([W1] * a + [W2] * b + [W3] * cc))
                    if key in seen:
                        continue
                    seen.add(key)
                    cost = (a * cost_fn(W1) + b * cost_fn(W2) +
                            (cc * cost_fn(W3) if W3 else 0))
                    configs.append((cost, W1, W2, W3, a, b, cc))
    configs.sort()
    best = None
    for cost, W1, W2, W3, a, b, cc in configs:
        pick = _solve_bundles3(nz, (W1, W2, W3),
                               (a * N_CORES, b * N_CORES, cc * N_CORES))
        if pick is not None:
            best = (cost, (W1, W2, W3, a, b, cc), pick)
            break
    if best is None:
        return None
    cost, (W1, W2, W3, a, b, cc), pick = best
    widths = (W1,) * a + (W2,) * b + ((W3,) * cc if W3 else ())
    slots = [[(core, s) for s in range(a) for core in range(N_CORES)],
             [(core, s + a) for s in range(b) for core in range(N_CORES)],
             [(core, s + a + b) for s in range(cc) for core in range(N_CORES)]]
    ptr = [0, 0, 0]
    Wv = (W1, W2, W3)
    assignment = [[None] * len(widths) for _ in range(N_CORES)]
    for (e, c), (w, ijk) in zip(nz, pick):
        start = 0
        rem = c
        for lvl in range(3):
            for _ in range(ijk[lvl]):
                core, s = slots[lvl][ptr[lvl]]; ptr[lvl] += 1
                n = min(rem, Wv[lvl])
                if n > 0:
                    assignment[core][s] = (e, start, n)
                start += n
                rem -= n
        assert rem == 0, (e, c, rem)
    return cost, widths, assignment


def _plan(counts, cost_fn):
    """Returns (widths, assignment): widths = per-core compiled chunk
    widths; assignment[core][slot] = (expert, start, fill) or None."""
    plans = [_plan_groupsort(counts, cost_fn)]
    tw = _plan_twowidth(counts, cost_fn)
    if tw is not None:
        plans.append(tw)
    plans.sort(key=lambda p: p[0])
    _, widths, assignment = plans[0]
    return tuple(widths), assignment


def _build_program(widths8, widths16):
    nc = bacc.Bacc("TRN2", debug=False, num_devices=N_CORES)

    ins = {}
    outs = {}

    def din(name, shape, dt):
        ins[name] = nc.dram_tensor(name, list(shape), dt, kind="ExternalInput").ap()
        return ins[name]

    def dout(name, shape, dt=F16):
        outs[name] = nc.dram_tensor(name, list(shape), dt, kind="ExternalOutput").ap()
        return outs[name]

    for s, w in enumerate(widths8):
        ntch = -(-w // 128)
        din(f"xg8_{s}", (D, w), F8)
        din(f"w1t8_{s}", (D, H), F8)
        din(f"w3t8_{s}", (D, H), F8)
        din(f"w2t8_{s}", (H, D), F8)
        din(f"b1_8_{s}", (128, HM), F32)
        din(f"b3_8_{s}", (128, HM), F32)
        din(f"scl8_{s}", (128, ntch), F32)
        dout(f"oe8_{s}", (ntch * 128, D))
    for s, w in enumerate(widths16):
        ntch = -(-w // 128)
        din(f"xg16_{s}", (D, w), F16)
        din(f"w1t16_{s}", (D, H), F16)
        din(f"w3t16_{s}", (D, H), F16)
        din(f"w2t16_{s}", (H, D), F16)
        din(f"b1_16_{s}", (128, HM), F32)
        din(f"b3_16_{s}", (128, HM), F32)
        din(f"scl16_{s}", (128, ntch), F32)
        dout(f"oe16_{s}", (ntch * 128, D))
    # shared expert: hidden-sharded (352 -> pad 384 per core), weights
    # SBUF-resident, 8 light token chunks interleaved as DMA relief
    din("xt", (D, T), F16)
    din("ws1s", (D, HS_PAD), F16)
    din("ws3s", (D, HS_PAD), F16)
    din("ws2sa", (HS_PAD, D), F16)
    din("bs1", (128, HMS), F32)
    din("bs3", (128, HMS), F32)
    dout("zs", (T, D))

    with tile.TileContext(nc) as tc:
        with (
            tc.tile_pool(name="xpool", bufs=2) as xpool,
            tc.tile_pool(name="hpool", bufs=2) as hpool,
            tc.tile_pool(name="wcol", bufs=3) as wcol,
            tc.tile_pool(name="w2pool", bufs=2) as w2pool,
            tc.tile_pool(name="tmp", bufs=2) as tmp,
            tc.tile_pool(name="opool", bufs=4) as opool,
            tc.tile_pool(name="cpool", bufs=1) as cpool,
            tc.tile_pool(name="pp", bufs=2, space="PSUM") as pp,
        ):
            def mlp_chunk8(s, w):
                """fp8 chunk: out[:w] = scl * (swiglu8(xg) @ W2^T); weights
                pre-scaled x64, dequant via act scale + scl (=cw/64)."""
                ntch = -(-w // 128)
                xg_ap = ins[f"xg8_{s}"]
                x3 = xg_ap.rearrange("(ko p) t -> p ko t", p=128)
                w1c3 = ins[f"w1t8_{s}"].rearrange("(ko p) h -> p ko h", p=128)
                w3c3 = ins[f"w3t8_{s}"].rearrange("(ko p) h -> p ko h", p=128)
                w23 = ins[f"w2t8_{s}"].rearrange("(k p) d -> p k d", p=128)
                out_ap = outs[f"oe8_{s}"]

                b1sb = cpool.tile([128, HM], F32, tag=f"b1f8{s}")
                nc.sync.dma_start(b1sb[:], ins[f"b1_8_{s}"])
                b3sb = cpool.tile([128, HM], F32, tag=f"b3f8{s}")
                nc.sync.dma_start(b3sb[:], ins[f"b3_8_{s}"])
                sclsb = cpool.tile([128, ntch], F32, tag=f"sclf8{s}")
                nc.sync.dma_start(sclsb[:], ins[f"scl8_{s}"])

                xsb = xpool.tile([128, KO, 512], F8, tag="xg8")
                nc.sync.dma_start(xsb[:, :, :w], x3)
                w2sbs = []
                hsb = hpool.tile([128, HM, 512], F8, tag="h8", bufs=2)
                for hm in range(HM):
                    if hm in (3, 8):
                        dm = 0 if hm == 3 else 1
                        w2sb = w2pool.tile([128, HM, 512], F8, tag="w2s8",
                                           bufs=2)
                        nc.sync.dma_start(
                            w2sb[:], w23[:, :, dm * 512:(dm + 1) * 512])
                        w2sbs.append(w2sb)
                    w1t_ = wcol.tile([128, KO, 128], F8, tag="w1c8", bufs=3)
                    nc.gpsimd.dma_start(w1t_[:], w1c3[:, :, hm * 128:(hm + 1) * 128])
                    w3t_ = wcol.tile([128, KO, 128], F8, tag="w3c8", bufs=3)
                    nc.gpsimd.dma_start(w3t_[:], w3c3[:, :, hm * 128:(hm + 1) * 128])
                    ps1 = pp.tile([128, 512], F32, tag="ph1")
                    for kk in range(KO // 2):
                        nc.tensor.matmul(ps1[:, :w],
                                         w1t_[:, 2 * kk:2 * kk + 2, :],
                                         xsb[:, 2 * kk:2 * kk + 2, :w],
                                         start=(kk == 0), stop=(kk == KO // 2 - 1),
                                         perf_mode=DR)
                    ps3 = pp.tile([128, 512], F32, tag="ph3")
                    for kk in range(KO // 2):
                        nc.tensor.matmul(ps3[:, :w],
                                         w3t_[:, 2 * kk:2 * kk + 2, :],
                                         xsb[:, 2 * kk:2 * kk + 2, :w],
                                         start=(kk == 0), stop=(kk == KO // 2 - 1),
                                         perf_mode=DR)
                    h1t = tmp.tile([128, 512], F32, tag="h1t")
                    nc.scalar.activation(h1t[:, :w], ps1[:, :w],
                                         mybir.ActivationFunctionType.Silu,
                                         bias=b1sb[:, hm:hm + 1],
                                         scale=1.0 / W8SCALE)
                    h3t = tmp.tile([128, 512], F32, tag="h3t")
                    nc.scalar.activation(h3t[:, :w], ps3[:, :w],
                                         mybir.ActivationFunctionType.Identity,
                                         bias=b3sb[:, hm:hm + 1],
                                         scale=1.0 / W8SCALE)
                    nc.vector.tensor_mul(hsb[:, hm, :w], h1t[:, :w], h3t[:, :w])
                # second matmul: out rows = tokens; 5 DoubleRow + 1 single
                for dm in range(4):
                    if dm == 1:
                        w2sb = w2pool.tile([128, HM, 512], F8, tag="w2s8",
                                           bufs=2)
                        nc.sync.dma_start(w2sb[:], w23[:, :, 2 * 512:3 * 512])
                        w2sbs.append(w2sb)
                    if dm == 2:
                        w2sb = w2pool.tile([128, HM, 512], F8, tag="w2s8",
                                           bufs=2)
                        nc.sync.dma_start(w2sb[:], w23[:, :, 3 * 512:4 * 512])
                        w2sbs.append(w2sb)
                    w2sb = w2sbs[dm]
                    for tch in range(ntch):
                        tok0 = tch * 128
                        tcw = min(128, w - tok0)
                        ps2 = pp.tile([128, 512], F32, tag="po", bufs=4)
                        for kk in range(5):
                            nc.tensor.matmul(
                                ps2[:tcw, :],
                                hsb[:, 2 * kk:2 * kk + 2, tok0:tok0 + tcw],
                                w2sb[:, 2 * kk:2 * kk + 2, :],
                                start=(kk == 0), stop=False, perf_mode=DR)
                        nc.tensor.matmul(ps2[:tcw, :],
                                         hsb[:, 10, tok0:tok0 + tcw],
                                         w2sb[:, 10, :],
                                         start=False, stop=True)
                        osb = opool.tile([128, 512], F16, tag="osb")
                        nc.vector.tensor_scalar_mul(
                            osb[:tcw, :], ps2[:tcw, :], sclsb[:tcw, tch:tch + 1])
                        nc.sync.dma_start(
                            out_ap[tok0:tok0 + tcw, dm * 512:(dm + 1) * 512],
                            osb[:tcw, :])

            def mlp_chunk16(xg_ap, w1_ap, w3_ap, w2_ap, b1_ap, b3_ap, scl_ap,
                            out_ap, w, n_hm, wtag):
                """fp16 chunk: out[:w] = scale * (swiglu(xg) @ W2^T)."""
                ntch = -(-w // 128)
                x3 = xg_ap.rearrange("(ko p) t -> p ko t", p=128)
                w1c3 = w1_ap.rearrange("(ko p) h -> p ko h", p=128)
                w3c3 = w3_ap.rearrange("(ko p) h -> p ko h", p=128)
                w23 = w2_ap.rearrange("(k p) d -> p k d", p=128)

                b1sb = cpool.tile([128, n_hm], F32, tag=f"b1{wtag}")
                nc.sync.dma_start(b1sb[:], b1_ap)
                b3sb = cpool.tile([128, n_hm], F32, tag=f"b3{wtag}")
                nc.sync.dma_start(b3sb[:], b3_ap)
                if scl_ap is not None:
                    sclsb = cpool.tile([128, ntch], F32, tag=f"scl{wtag}")
                    nc.sync.dma_start(sclsb[:], scl_ap)

                xsb = xpool.tile([128, KO, 512], F16, tag="xg16")
                nc.sync.dma_start(xsb[:, :, :w], x3)
                w2sbs = []
                hsb = hpool.tile([128, n_hm, 512], F16, tag=f"h16_{n_hm}",
                                 bufs=2)
                for hm in range(n_hm):
                    if hm in (3, 8):
                        dm = 0 if hm == 3 else 1
                        w2sb = w2pool.tile([128, n_hm, 512], F16,
                                           tag=f"w2s16_{n_hm}", bufs=2)
                        nc.sync.dma_start(
                            w2sb[:], w23[:, :, dm * 512:(dm + 1) * 512])
                        w2sbs.append(w2sb)
                    w1t_ = wcol.tile([128, KO, 128], F16, tag="w1c16", bufs=3)
                    nc.gpsimd.dma_start(w1t_[:], w1c3[:, :, hm * 128:(hm + 1) * 128])
                    w3t_ = wcol.tile([128, KO, 128], F16, tag="w3c16", bufs=3)
                    nc.gpsimd.dma_start(w3t_[:], w3c3[:, :, hm * 128:(hm + 1) * 128])
                    ps1 = pp.tile([128, 512], F32, tag="ph1")
                    for ko in range(KO):
                        nc.tensor.matmul(ps1[:, :w], w1t_[:, ko, :], xsb[:, ko, :w],
                                         start=(ko == 0), stop=(ko == KO - 1))
                    ps3 = pp.tile([128, 512], F32, tag="ph3")
                    for ko in range(KO):
                        nc.tensor.matmul(ps3[:, :w], w3t_[:, ko, :], xsb[:, ko, :w],
                                         start=(ko == 0), stop=(ko == KO - 1))
                    h1t = tmp.tile([128, 512], F32, tag="h1t")
                    nc.scalar.activation(h1t[:, :w], ps1[:, :w],
                                         mybir.ActivationFunctionType.Silu,
                                         bias=b1sb[:, hm:hm + 1])
                    h3t = tmp.tile([128, 512], F32, tag="h3t")
                    nc.scalar.activation(h3t[:, :w], ps3[:, :w],
                                         mybir.ActivationFunctionType.Identity,
                                         bias=b3sb[:, hm:hm + 1])
                    nc.vector.tensor_mul(hsb[:, hm, :w], h1t[:, :w], h3t[:, :w])
                # second matmul: out rows = tokens
                for dm in range(4):
                    if n_hm > 4 and dm in (1, 2):
                        w2sb = w2pool.tile([128, n_hm, 512], F16,
                                           tag=f"w2s16_{n_hm}", bufs=2)
                        nc.sync.dma_start(
                            w2sb[:], w23[:, :, (dm + 1) * 512:(dm + 2) * 512])
                        w2sbs.append(w2sb)
                    w2sb = w2sbs[dm]
                    for tch in range(ntch):
                        tok0 = tch * 128
                        tcw = min(128, w - tok0)
                        ps2 = pp.tile([128, 512], F32, tag="po", bufs=4)
                        for k in range(n_hm):
                            lhsT = hsb[:, k, tok0:tok0 + tcw]
                            nc.tensor.matmul(ps2[:tcw, :], lhsT, w2sb[:, k, :],
                                             start=(k == 0), stop=(k == n_hm - 1))
                        osb = opool.tile([128, 512], F16, tag="osb")
                        if scl_ap is not None:
                            nc.vector.tensor_scalar_mul(
                                osb[:tcw, :], ps2[:tcw, :], sclsb[:tcw, tch:tch + 1])
                        else:
                            nc.vector.tensor_copy(osb[:tcw, :], ps2[:tcw, :])
                        nc.sync.dma_start(
                            out_ap[tok0:tok0 + tcw, dm * 512:(dm + 1) * 512],
                            osb[:tcw, :])

            # shared-expert residents (emitted piecewise between early chunks)
            ws1r = cpool.tile([128, KO, HS_PAD], F16, tag="ws1r")
            ws3r = cpool.tile([128, KO, HS_PAD], F16, tag="ws3r")
            ws2r = cpool.tile([128, HMS, D], F16, tag="ws2r")
            bs1r = cpool.tile([128, HMS], F32, tag="bs1r")
            bs3r = cpool.tile([128, HMS], F32, tag="bs3r")
            resident_loads = [
                lambda: nc.sync.dma_start(
                    ws1r[:], ins["ws1s"].rearrange("(ko p) h -> p ko h", p=128)),
                lambda: nc.sync.dma_start(
                    ws3r[:], ins["ws3s"].rearrange("(ko p) h -> p ko h", p=128)),
                lambda: (nc.sync.dma_start(
                    ws2r[:], ins["ws2sa"].rearrange("(k p) d -> p k d", p=128)),
                    nc.sync.dma_start(bs1r[:], ins["bs1"]),
                    nc.sync.dma_start(bs3r[:], ins["bs3"])),
            ]

            def routed_chunk16(s, w):
                mlp_chunk16(ins[f"xg16_{s}"], ins[f"w1t16_{s}"], ins[f"w3t16_{s}"],
                            ins[f"w2t16_{s}"], ins[f"b1_16_{s}"], ins[f"b3_16_{s}"],
                            ins[f"scl16_{s}"], outs[f"oe16_{s}"], w, HM, f"e{s}")

            def shared_chunk(t):
                xt3 = ins["xt"].rearrange("(ko p) t -> p ko t", p=128)
                xsb = xpool.tile([128, KO, 512], F16, tag="xg16")
                nc.sync.dma_start(xsb[:], xt3[:, :, t * 512:(t + 1) * 512])
                hsb = hpool.tile([128, HM, 512], F16, tag=f"h16_{HM}", bufs=2)
                for hm in range(HMS):
                    ps1 = pp.tile([128, 512], F32, tag="ph1")
                    for ko in range(KO):
                        nc.tensor.matmul(ps1[:], ws1r[:, ko, hm * 128:(hm + 1) * 128],
                                         xsb[:, ko, :],
                                         start=(ko == 0), stop=(ko == KO - 1))
                    ps3 = pp.tile([128, 512], F32, tag="ph3")
                    for ko in range(KO):
                        nc.tensor.matmul(ps3[:], ws3r[:, ko, hm * 128:(hm + 1) * 128],
                                         xsb[:, ko, :],
                                         start=(ko == 0), stop=(ko == KO - 1))
                    h1t = tmp.tile([128, 512], F32, tag="h1t")
                    nc.scalar.activation(h1t[:], ps1[:],
                                         mybir.ActivationFunctionType.Silu,
                                         bias=bs1r[:, hm:hm + 1])
                    h3t = tmp.tile([128, 512], F32, tag="h3t")
                    nc.scalar.activation(h3t[:], ps3[:],
                                         mybir.ActivationFunctionType.Identity,
                                         bias=bs3r[:, hm:hm + 1])
                    nc.vector.tensor_mul(hsb[:, hm, :], h1t[:], h3t[:])
                for dm in range(4):
                    for tch in range(4):
                        tok0 = t * 512 + tch * 128
                        ps2 = pp.tile([128, 512], F32, tag="po", bufs=4)
                        for k in range(HMS):
                            lhsT = hsb[:, k, tch * 128:(tch + 1) * 128]
                            nc.tensor.matmul(ps2[:], lhsT,
                                             ws2r[:, k, dm * 512:(dm + 1) * 512],
                                             start=(k == 0), stop=(k == HMS - 1))
                        osb = opool.tile([128, 512], F16, tag="osb")
                        nc.vector.tensor_copy(osb[:], ps2[:])
                        nc.sync.dma_start(
                            outs["zs"][tok0:tok0 + 128, dm * 512:(dm + 1) * 512],
                            osb[:])

            # routed order: spread fp16 chunks evenly among fp8 chunks
            n8, n16 = len(widths8), len(widths16)
            routed = []
            i8 = i16 = 0
            while i8 < n8 or i16 < n16:
                if i8 < n8:
                    routed.append(("r8", i8)); i8 += 1
                while i16 < n16 and (i8 >= n8 or (i16 + 1) * n8 <= (i8) * n16 + n16 // 2):
                    routed.append(("r16", i16)); i16 += 1
            # interleave shared chunks to smooth DMA
            n_r, n_s = len(routed), T // 512
            ri, si = 0, 0
            seq = []
            while ri < n_r or si < n_s:
                if ri < n_r:
                    seq.append(routed[ri]); ri += 1
                while si < n_s and (ri >= n_r or (si + 1) * n_r <= (ri + 1) * n_s):
                    seq.append(("s", si)); si += 1
            for n_emitted, (kind, i) in enumerate(seq):
                if kind == "r8":
                    mlp_chunk8(i, widths8[i])
                elif kind == "r16":
                    routed_chunk16(i, widths16[i])
                else:
                    shared_chunk(i)
                if n_emitted == 0:
                    for ld in resident_loads:
                        ld()

    nc.compile()
    return nc


def kernel(x, gate_w, gate_b, w1, b1, w2, b2, w3, b3,
           ws1, bs1, ws2, bs2, ws3, bs3):
    F8NP = _np_f8()
    x = np.asarray(x, np.float32)
    xf = np.ascontiguousarray(x.reshape(-1, D))
    gate_w = np.asarray(gate_w, np.float32)
    gate_b = np.asarray(gate_b, np.float32)
    w1 = np.asarray(w1, np.float32)
    b1 = np.asarray(b1, np.float32)
    w2 = np.asarray(w2, np.float32)
    b2 = np.asarray(b2, np.float32)
    w3 = np.asarray(w3, np.float32)
    b3 = np.asarray(b3, np.float32)
    ws1 = np.asarray(ws1, np.float32)
    bs1 = np.asarray(bs1, np.float32)
    ws2 = np.asarray(ws2, np.float32)
    bs2 = np.asarray(bs2, np.float32)
    ws3 = np.asarray(ws3, np.float32)
    bs3 = np.asarray(bs3, np.float32)

    cw, mask = _host_gate(xf, gate_w, gate_b)
    hi = mask & (cw >= TAU)
    lo = mask & (cw < TAU)
    toks8 = [np.flatnonzero(lo[:, e]).astype(np.int64) for e in range(E)]
    toks16 = [np.flatnonzero(hi[:, e]).astype(np.int64) for e in range(E)]
    counts8 = np.array([len(t) for t in toks8])
    counts16 = np.array([len(t) for t in toks16])
    widths8, asg8 = _plan(counts8, _chunk_cost8)
    widths16, asg16 = _plan(counts16, _chunk_cost)

    key = (widths8, widths16)
    if key not in _PROGRAM_CACHE:
        _PROGRAM_CACHE[key] = _build_program(widths8, widths16)
    nc = _PROGRAM_CACHE[key]

    xT = np.ascontiguousarray(xf.T)  # [D, T]
    xT16 = xT.astype(np.float16)
    xT8 = xT.astype(F8NP)

    w1t8, w3t8, w2t8 = {}, {}, {}
    w1t16, w3t16, w2t16 = {}, {}, {}
    b1t, b3t = {}, {}

    def prep_bias(e):
        if e not in b1t:
            b1t[e] = np.ascontiguousarray(b1[e].reshape(HM, 128).T)
            b3t[e] = np.ascontiguousarray(b3[e].reshape(HM, 128).T)

    need8 = sorted({p[0] for slots in asg8 for p in slots if p is not None})
    need16 = sorted({p[0] for slots in asg16 for p in slots if p is not None})
    for e in need8:
        w1t8[e] = np.ascontiguousarray((w1[e].T * W8SCALE)).astype(F8NP)
        w3t8[e] = np.ascontiguousarray((w3[e].T * W8SCALE)).astype(F8NP)
        w2t8[e] = np.ascontiguousarray((w2[e].T * W8SCALE)).astype(F8NP)
        prep_bias(e)
    for e in need16:
        w1t16[e] = np.ascontiguousarray(w1[e].T).astype(np.float16)
        w3t16[e] = np.ascontiguousarray(w3[e].T).astype(np.float16)
        w2t16[e] = np.ascontiguousarray(w2[e].T).astype(np.float16)
        prep_bias(e)

    hs_per = HS // N_CORES  # 352

    in_maps = []
    for c in range(N_CORES):
        m = {}
        for s, w in enumerate(widths8):
            ntch = -(-w // 128)
            piece = asg8[c][s]
            xg = np.zeros((D, w), F8NP)
            scl = np.zeros(ntch * 128, np.float32)
            if piece is None:
                e = need8[0]
            else:
                e, s0, n = piece
                tk = toks8[e][s0:s0 + n]
                xg[:, :n] = xT8[:, tk]
                scl[:n] = cw[tk, e] / W8SCALE
            m[f"w1t8_{s}"] = w1t8[e]
            m[f"w3t8_{s}"] = w3t8[e]
            m[f"w2t8_{s}"] = w2t8[e]
            m[f"b1_8_{s}"] = b1t[e]
            m[f"b3_8_{s}"] = b3t[e]
            m[f"xg8_{s}"] = xg
            m[f"scl8_{s}"] = np.ascontiguousarray(scl.reshape(ntch, 128).T)
        for s, w in enumerate(widths16):
            ntch = -(-w // 128)
            piece = asg16[c][s]
            xg = np.zeros((D, w), np.float16)
            scl = np.zeros(ntch * 128, np.float32)
            if piece is None:
                e = need16[0]
            else:
                e, s0, n = piece
                tk = toks16[e][s0:s0 + n]
                xg[:, :n] = xT16[:, tk]
                scl[:n] = cw[tk, e]
            m[f"w1t16_{s}"] = w1t16[e]
            m[f"w3t16_{s}"] = w3t16[e]
            m[f"w2t16_{s}"] = w2t16[e]
            m[f"b1_16_{s}"] = b1t[e]
            m[f"b3_16_{s}"] = b3t[e]
            m[f"xg16_{s}"] = xg
            m[f"scl16_{s}"] = np.ascontiguousarray(scl.reshape(ntch, 128).T)
        # shared expert shard (352 hidden rows, padded to 384)
        r0 = c * hs_per
        ws1p = np.zeros((D, HS_PAD), np.float16)
        ws1p[:, :hs_per] = ws1[r0:r0 + hs_per].T
        ws3p = np.zeros((D, HS_PAD), np.float16)
        ws3p[:, :hs_per] = ws3[r0:r0 + hs_per].T
        ws2a = np.zeros((HS_PAD, D), np.float16)
        ws2a[:hs_per] = ws2[:, r0:r0 + hs_per].T
        bs1p = np.zeros(HS_PAD, np.float32)
        bs1p[:hs_per] = bs1[r0:r0 + hs_per]
        bs3p = np.zeros(HS_PAD, np.float32)
        bs3p[:hs_per] = bs3[r0:r0 + hs_per]
        m["xt"] = xT16
        m["ws1s"] = ws1p
        m["ws3s"] = ws3p
        m["ws2sa"] = ws2a
        m["bs1"] = np.ascontiguousarray(bs1p.reshape(HMS, 128).T)
        m["bs3"] = np.ascontiguousarray(bs3p.reshape(HMS, 128).T)
        in_maps.append(m)

    res = run_bass_kernel_spmd(nc, in_maps, list(range(N_CORES)))

    y = np.zeros((T, D), np.float32)
    for c in range(N_CORES):
        for s, w in enumerate(widths8):
            piece = asg8[c][s]
            if piece is None:
                continue
            e, s0, n = piece
            tk = toks8[e][s0:s0 + n]
            y[tk] += res.results[c][f"oe8_{s}"][:n].astype(np.float32)
            y[tk] += cw[tk, e][:, None] * b2[e][None, :]
        for s, w in enumerate(widths16):
            piece = asg16[c][s]
            if piece is None:
                continue
            e, s0, n = piece
            tk = toks16[e][s0:s0 + n]
            y[tk] += res.results[c][f"oe16_{s}"][:n].astype(np.float32)
            y[tk] += cw[tk, e][:, None] * b2[e][None, :]
        y += res.results[c]["zs"].astype(np.float32)
    y += bs2[None, :]
    return y.reshape(x.shape).astype(np.float32)
